# revision 6
# baseline (speedup 1.0000x reference)
"""BiFormer sparse-attention kernel for 8 Trainium2 NeuronCores.

Layout: core c handles batch b=c//2, query half h=c%2 (2048 queries each).
Keys for each core are column-permuted so its own 2048 queries occupy
positions 0:2048 (key permutation is harmless: top-k/softmax/ctx are
permutation-invariant over keys).
"""

import math
from contextlib import ExitStack

import numpy as np

import concourse.bass as bass
import concourse.mybir as mybir
import concourse.tile as tile
from concourse import bacc
from concourse.bass_utils import run_bass_kernel_spmd

# ---- problem constants (hardcoded per task contract) ----
B, N, F, A = 4, 4096, 15, 24
HID, HD, AFN = 10, 10, 14
KS = [2, 3, 4, 5, 6, 7]
FN = [3, 3, 3, 2, 2, 1]
K = 204                       # int(N * 0.05)
SCALE = float(np.float32(1.0 / math.sqrt(HD)))
NQ = N // 2                   # queries per core
QT = NQ // 128                # query tiles per core (16)
CH = 32                       # key chunks for candidate extraction
CHL = N // CH                 # chunk length (128)
NCAND = CH * 8                # candidate count (256)
NSORT_IT = (K + 7) // 8       # 26 -> extracts 208 sorted values
XROWS = F * A                 # 360
XR_PAD = 363                  # 3 chunks of 121 partitions
GCOL = 24                     # padded positions per conv group
NGRP = AFN                    # 14 output channels
WB_COLS = NGRP * GCOL         # 336
NEG = -1.0e9

_dt = mybir.dt


def _build_wbig(inputs):
    """Block-Toeplitz conv weights [363, 336]; row 360 = const-1 row used to
    kill padded positions with NEG; rows 361/362 zero padding."""
    wb = np.zeros((XR_PAD, WB_COLS), np.float32)
    g = 0
    for h, f in zip(KS, FN):
        w = inputs["conv_w%d" % h]  # [f, F, h]
        npos = A - h + 1
        for o in range(f):
            base = g * GCOL
            for p in range(npos):
                for c in range(F):
                    for j in range(h):
                        wb[c * A + p + j, base + p] += w[o, c, j]
            wb[XROWS, base + npos:base + GCOL] = NEG
            g += 1
    assert g == NGRP
    return wb


def build_nc():
    nc = bacc.Bacc("TRN2", target_bir_lowering=False, debug=False,
                   enable_asserts=False, num_devices=8)
    f32 = _dt.float32

    xT_d = nc.dram_tensor("xT", [XR_PAD, N], f32, kind="ExternalInput").ap()
    wbig_d = nc.dram_tensor("wbig", [XR_PAD, WB_COLS], f32, kind="ExternalInput").ap()
    wq15_d = nc.dram_tensor("wq15", [15, HID], f32, kind="ExternalInput").ap()
    wk15_d = nc.dram_tensor("wk15", [15, HID], f32, kind="ExternalInput").ap()
    wv15_d = nc.dram_tensor("wv15", [15, HID + 1], f32, kind="ExternalInput").ap()
    bias14_d = nc.dram_tensor("bias14", [128, NGRP], f32, kind="ExternalInput").ap()
    ident_d = nc.dram_tensor("ident", [128, 128], f32, kind="ExternalInput").ap()
    ones_d = nc.dram_tensor("ones_row", [1, N], f32, kind="ExternalInput").ap()

    probs_out = nc.dram_tensor("probs_out", [NQ, K], f32, kind="ExternalOutput").ap()
    acc_out = nc.dram_tensor("acc_out", [128, HID], f32, kind="ExternalOutput").ap()

    KT = N // 128  # 32 key tiles

    with tile.TileContext(nc) as tc, ExitStack() as ctx:
        # ---- long-lived SBUF ----
        persist = ctx.enter_context(tc.tile_pool(name="persist", bufs=1))
        featsT = persist.tile([15, N], f32, tag="featsT")
        qTt = persist.tile([HID, NQ], f32, tag="qT")
        kTt = persist.tile([HID, N], f32, tag="kT")
        vA = persist.tile([128, KT, HID + 1], f32, tag="vA")
        acc = persist.tile([128, HID], f32, tag="acc")
        wbig_s = persist.tile([121, 3, WB_COLS], f32, tag="wbig")
        wq_s = persist.tile([15, HID], f32, tag="wq")
        wk_s = persist.tile([15, HID], f32, tag="wk")
        wv_s = persist.tile([15, HID + 1], f32, tag="wv")
        b14_s = persist.tile([128, NGRP], f32, tag="b14")
        id_s = persist.tile([128, 128], f32, tag="id")

        nc.sync.dma_start(wbig_s[:], wbig_d.rearrange("(c p) f -> p c f", c=3))
        nc.sync.dma_start(wq_s[:], wq15_d[:])
        nc.sync.dma_start(wk_s[:], wk15_d[:])
        nc.sync.dma_start(wv_s[:], wv15_d[:])
        nc.sync.dma_start(b14_s[:], bias14_d[:])
        nc.sync.dma_start(id_s[:], ident_d[:])
        nc.sync.dma_start(featsT[14:15, :], ones_d[:])
        nc.vector.memset(acc[:], 0.0)

        xT_view = xT_d.rearrange("(c p) n -> p c n", c=3)

        # ---- stage 1: features for all N keys ----
        with tc.tile_pool(name="f_sb", bufs=3) as fsb, \
             tc.tile_pool(name="f_ps", bufs=2, space="PSUM") as fps, \
             tc.tile_pool(name="f_ps2", bufs=2, space="PSUM") as fps2:
            for t in range(KT):
                xt = fsb.tile([121, 3, 128], f32, tag="xt")
                nc.sync.dma_start(xt[:], xT_view[:, :, t * 128:(t + 1) * 128])
                psF = fps.tile([128, WB_COLS], f32, tag="psF")
                for c in range(3):
                    nc.tensor.matmul(psF[:], xt[:, c, :], wbig_s[:, c, :],
                                     start=(c == 0), stop=(c == 2))
                pooled = fsb.tile([128, NGRP], f32, tag="pooled")
                nc.vector.tensor_reduce(
                    pooled[:], psF[:].rearrange("p (g w) -> p g w", w=GCOL),
                    axis=mybir.AxisListType.X, op=mybir.AluOpType.max)
                feats = fsb.tile([128, NGRP], f32, tag="feats")
                nc.vector.tensor_add(feats[:], pooled[:], b14_s[:])
                nc.vector.tensor_scalar_max(feats[:], feats[:], 0.0)
                psT = fps2.tile([NGRP, 128], f32, tag="psT")
                nc.tensor.transpose(psT[:], feats[:], id_s[:])
                nc.scalar.copy(featsT[0:NGRP, t * 128:(t + 1) * 128], psT[:])

        # ---- stage 2: qT, kT, V ----
        with tc.tile_pool(name="q_ps", bufs=2, space="PSUM") as qps, \
             tc.tile_pool(name="v_ps", bufs=2, space="PSUM") as vps:
            for c in range(NQ // 512):
                ps = qps.tile([HID, 512], f32, tag="psQ")
                nc.tensor.matmul(ps[:], wq_s[:], featsT[:, c * 512:(c + 1) * 512],
                                 start=True, stop=True)
                nc.scalar.copy(qTt[:, c * 512:(c + 1) * 512], ps[:])
            for c in range(N // 512):
                ps = qps.tile([HID, 512], f32, tag="psQ")
                nc.tensor.matmul(ps[:], wk_s[:], featsT[:, c * 512:(c + 1) * 512],
                                 start=True, stop=True)
                nc.scalar.copy(kTt[:, c * 512:(c + 1) * 512], ps[:])
            for c in range(KT):
                ps = vps.tile([128, HID + 1], f32, tag="psV")
                nc.tensor.matmul(ps[:], featsT[:, c * 128:(c + 1) * 128], wv_s[:],
                                 start=True, stop=True)
                nc.scalar.copy(vA[:, c, :], ps[:])

        # ---- stage 3: attention per query tile ----
        with tc.tile_pool(name="a_sb", bufs=2) as asb, \
             tc.tile_pool(name="a_small", bufs=2) as asm, \
             tc.tile_pool(name="pt_sb", bufs=3) as ptsb, \
             tc.tile_pool(name="s_ps", bufs=3, space="PSUM") as sps, \
             tc.tile_pool(name="t_ps", bufs=2, space="PSUM") as tps, \
             tc.tile_pool(name="c_ps", bufs=2, space="PSUM") as cps:
            for t in range(QT):
                qs = t * 128
                E = asb.tile([128, N], f32, tag="E")
                for c in range(N // 512):
                    psS = sps.tile([128, 512], f32, tag="psS")
                    nc.tensor.matmul(psS[:], qTt[:, qs:qs + 128],
                                     kTt[:, c * 512:(c + 1) * 512],
                                     start=True, stop=True)
                    nc.scalar.activation(E[:, c * 512:(c + 1) * 512], psS[:],
                                         mybir.ActivationFunctionType.Exp,
                                         scale=SCALE)

                # candidates: top-8 of each 128-key chunk
                candA = asm.tile([128, NCAND], f32, tag="candA")
                candB = asm.tile([128, NCAND], f32, tag="candB")
                for c in range(CH):
                    nc.vector.max(candA[:, c * 8:(c + 1) * 8],
                                  E[:, c * CHL:(c + 1) * CHL])

                # selection sort: 26 x (max8 + match_replace)
                srt = asm.tile([128, NSORT_IT * 8], f32, tag="srt")
                bufs = [candA, candB]
                for i in range(NSORT_IT):
                    src = bufs[i % 2]
                    dst = bufs[(i + 1) % 2]
                    nc.vector.max(srt[:, i * 8:(i + 1) * 8], src[:])
                    if i != NSORT_IT - 1:
                        nc.vector.match_replace(dst[:], srt[:, i * 8:(i + 1) * 8],
                                                src[:], 0.0)

                # probs = sorted / sum(sorted[:204])
                zs = asm.tile([128, 1], f32, tag="zs")
                rz = asm.tile([128, 1], f32, tag="rz")
                nc.vector.tensor_reduce(zs[:], srt[:, 0:K],
                                        axis=mybir.AxisListType.X,
                                        op=mybir.AluOpType.add)
                nc.vector.reciprocal(rz[:], zs[:])
                probs = asm.tile([128, K], f32, tag="probs")
                nc.scalar.activation(probs[:], srt[:, 0:K],
                                     mybir.ActivationFunctionType.Copy,
                                     scale=rz[:])
                nc.sync.dma_start(probs_out[qs:qs + 128, :], probs[:])

                # dense masked weights P = E * (E >= tau)
                P = asb.tile([128, N], f32, tag="P")
                nc.vector.scalar_tensor_tensor(
                    P[:], E[:], srt[:, K - 1:K], E[:],
                    op0=mybir.AluOpType.is_ge, op1=mybir.AluOpType.mult)

                # ctx = P @ V_aug via 32 transposed chunks
                psC = cps.tile([128, HID + 1], f32, tag="psC")
                for c in range(KT):
                    psT2 = tps.tile([128, 128], f32, tag="psT2")
                    nc.tensor.transpose(psT2[:], P[:, c * 128:(c + 1) * 128], id_s[:])
                    pts = ptsb.tile([128, 128], f32, tag="pts")
                    nc.scalar.copy(pts[:], psT2[:])
                    nc.tensor.matmul(psC[:], pts[:], vA[:, c, :],
                                     start=(c == 0), stop=(c == KT - 1))
                rzd = asm.tile([128, 1], f32, tag="rzd")
                nc.vector.reciprocal(rzd[:], psC[:, HID:HID + 1])
                ctxn = asm.tile([128, HID], f32, tag="ctxn")
                nc.scalar.activation(ctxn[:], psC[:, 0:HID],
                                     mybir.ActivationFunctionType.Copy,
                                     scale=rzd[:])
                nc.vector.tensor_add(acc[:], acc[:], ctxn[:])

            nc.sync.dma_start(acc_out[:], acc[:])

    nc.compile()
    return nc


_NC_CACHE = None


def _get_nc():
    global _NC_CACHE
    if _NC_CACHE is None:
        _NC_CACHE = build_nc()
    return _NC_CACHE


def _host_inputs(inputs):
    """Build per-core input maps."""
    x = np.asarray(inputs["x"], np.float32)
    wbig = _build_wbig(inputs)

    def wxx15(w, b, extra_ones=False):
        o = w.shape[0]
        cols = o + (1 if extra_ones else 0)
        m = np.zeros((15, cols), np.float32)
        m[0:AFN, 0:o] = np.asarray(w, np.float32).T
        m[14, 0:o] = np.asarray(b, np.float32)
        if extra_ones:
            m[14, o] = 1.0
        return m

    wq15 = wxx15(inputs["w_q"], inputs["b_q"])
    wk15 = wxx15(inputs["w_k"], inputs["b_k"])
    wv15 = wxx15(inputs["w_v"], inputs["b_v"], extra_ones=True)
    bias14 = np.broadcast_to(
        np.concatenate([np.asarray(inputs["conv_b%d" % h], np.float32)
                        for h in KS])[None, :], (128, NGRP)).copy()
    ident = np.eye(128, dtype=np.float32)

    in_maps = []
    for core in range(8):
        b, h = core // 2, core % 2
        xb = x[b].reshape(N, XROWS)
        if h == 1:
            xb = np.concatenate([xb[NQ:], xb[:NQ]], axis=0)
        xT = np.empty((XR_PAD, N), np.float32)
        xT[0:XROWS] = xb.T
        xT[XROWS] = 1.0
        xT[XROWS + 1:] = 0.0
        in_maps.append({
            "xT": np.ascontiguousarray(xT),
            "wbig": wbig, "wq15": wq15, "wk15": wk15, "wv15": wv15,
            "bias14": bias14, "ident": ident,
            "ones_row": np.ones((1, N), np.float32),
        })
    return in_maps


def kernel(**inputs):
    nc = _get_nc()
    in_maps = _host_inputs(inputs)
    res = run_bass_kernel_spmd(nc, in_maps, core_ids=list(range(8)))
    return _assemble(inputs, res.results)


def _assemble(inputs, results):
    probs = np.empty((B, N, K), np.float32)
    ctx_sum = np.zeros((B, HID), np.float64)
    for core in range(8):
        b, h = core // 2, core % 2
        r = results[core]
        probs[b, h * NQ:(h + 1) * NQ] = r["probs_out"]
        ctx_sum[b] += r["acc_out"].sum(axis=0, dtype=np.float64)

    w_attn = np.asarray(inputs["w_attn"], np.float64)
    b_attn = np.asarray(inputs["b_attn"], np.float64)
    w_mil = np.asarray(inputs["w_mil"], np.float64)
    b_mil = np.asarray(inputs["b_mil"], np.float64)
    wc = w_mil @ w_attn                     # [2, 10]
    bc = b_attn @ w_mil.T + b_mil           # [2]
    pooled = (ctx_sum @ wc.T) / N + bc
    return (pooled.astype(np.float32),
            probs.reshape(B, 1, N, 1, K).astype(np.float32))


# revision 10
# speedup vs baseline: 1.2079x; 1.2079x over previous
"""BiFormer sparse-attention kernel for 8 Trainium2 NeuronCores.

Layout: core c handles batch b=c//2, query half h=c%2 (2048 queries each).
Keys for each core are column-permuted so its own 2048 queries occupy
positions 0:2048 (key permutation is harmless: top-k/softmax/ctx are
permutation-invariant over keys).
"""

import math
from contextlib import ExitStack

import numpy as np

import concourse.bass as bass
import concourse.mybir as mybir
import concourse.tile as tile
from concourse import bacc
from concourse.bass_utils import run_bass_kernel_spmd

# ---- problem constants (hardcoded per task contract) ----
B, N, F, A = 4, 4096, 15, 24
HID, HD, AFN = 10, 10, 14
KS = [2, 3, 4, 5, 6, 7]
FN = [3, 3, 3, 2, 2, 1]
K = 204                       # int(N * 0.05)
SCALE = float(np.float32(1.0 / math.sqrt(HD)))
NQ = N // 2                   # queries per core
QT = NQ // 128                # query tiles per core (16)
CH = 32                       # key chunks for candidate extraction
CHL = N // CH                 # chunk length (128)
NCAND = CH * 8                # candidate count (256)
NSORT_IT = (K + 7) // 8       # 26 -> extracts 208 sorted values
XROWS = F * A                 # 360
XR_PAD = 363                  # 3 chunks of 121 partitions
GCOL = 24                     # padded positions per conv group
NGRP = AFN                    # 14 output channels
WB_COLS = NGRP * GCOL         # 336
NEG = -1.0e9

_dt = mybir.dt


def _build_wbig(inputs):
    """Block-Toeplitz conv weights [363, 336]; row 360 = const-1 row used to
    kill padded positions with NEG; rows 361/362 zero padding."""
    wb = np.zeros((XR_PAD, WB_COLS), np.float32)
    g = 0
    for h, f in zip(KS, FN):
        w = inputs["conv_w%d" % h]  # [f, F, h]
        npos = A - h + 1
        for o in range(f):
            base = g * GCOL
            for p in range(npos):
                for c in range(F):
                    for j in range(h):
                        wb[c * A + p + j, base + p] += w[o, c, j]
            wb[XROWS, base + npos:base + GCOL] = NEG
            g += 1
    assert g == NGRP
    return wb


def build_nc():
    nc = bacc.Bacc("TRN2", target_bir_lowering=False, debug=False,
                   enable_asserts=False, num_devices=8)
    f32 = _dt.float32

    xT_d = nc.dram_tensor("xT", [XR_PAD, N], f32, kind="ExternalInput").ap()
    wbig_d = nc.dram_tensor("wbig", [XR_PAD, WB_COLS], f32, kind="ExternalInput").ap()
    wq15_d = nc.dram_tensor("wq15", [15, HID], f32, kind="ExternalInput").ap()
    wk15_d = nc.dram_tensor("wk15", [15, HID], f32, kind="ExternalInput").ap()
    wv15_d = nc.dram_tensor("wv15", [15, HID + 1], f32, kind="ExternalInput").ap()
    bias14_d = nc.dram_tensor("bias14", [128, NGRP], f32, kind="ExternalInput").ap()
    ident_d = nc.dram_tensor("ident", [128, 128], f32, kind="ExternalInput").ap()
    ones_d = nc.dram_tensor("ones_row", [1, N], f32, kind="ExternalInput").ap()
    iota_d = nc.dram_tensor("iota_row", [128, 256], f32, kind="ExternalInput").ap()

    probs_out = nc.dram_tensor("probs_out", [NQ, K], f32, kind="ExternalOutput").ap()
    acc_out = nc.dram_tensor("acc_out", [128, HID], f32, kind="ExternalOutput").ap()

    KT = N // 128  # 32 key tiles
    f32r = _dt.float32r
    bf16 = _dt.bfloat16

    with tile.TileContext(nc) as tc, ExitStack() as ctx:
        # ---- long-lived SBUF ----
        persist = ctx.enter_context(tc.tile_pool(name="persist", bufs=1))
        featsT = persist.tile([15, N], f32, tag="featsT")
        qTt = persist.tile([HID, NQ], f32, tag="qT")
        kTt = persist.tile([HID, N], f32, tag="kT")
        vB = persist.tile([128, KT, HID + 1], bf16, tag="vB")
        iota_s = persist.tile([128, 256], f32, tag="iota")
        acc = persist.tile([128, HID], f32, tag="acc")
        wbig_s = persist.tile([121, 3, WB_COLS], f32, tag="wbig")
        wq_s = persist.tile([15, HID], f32, tag="wq")
        wk_s = persist.tile([15, HID], f32, tag="wk")
        wv_s = persist.tile([15, HID + 1], f32, tag="wv")
        b14_s = persist.tile([128, NGRP], f32, tag="b14")
        id_s = persist.tile([128, 128], f32, tag="id")

        nc.sync.dma_start(wbig_s[:], wbig_d.rearrange("(c p) f -> p c f", c=3))
        nc.sync.dma_start(wq_s[:], wq15_d[:])
        nc.sync.dma_start(wk_s[:], wk15_d[:])
        nc.sync.dma_start(wv_s[:], wv15_d[:])
        nc.sync.dma_start(b14_s[:], bias14_d[:])
        nc.sync.dma_start(id_s[:], ident_d[:])
        nc.sync.dma_start(iota_s[:], iota_d[:])
        nc.sync.dma_start(featsT[14:15, :], ones_d[:])
        nc.vector.memset(acc[:], 0.0)

        xT_view = xT_d.rearrange("(c p) n -> p c n", c=3)

        # ---- stage 1: features for all N keys ----
        with tc.tile_pool(name="f_sb", bufs=3) as fsb, \
             tc.tile_pool(name="f_ps", bufs=2, space="PSUM") as fps, \
             tc.tile_pool(name="f_ps2", bufs=2, space="PSUM") as fps2:
            for t in range(KT):
                xt = fsb.tile([121, 3, 128], f32, tag="xt")
                nc.sync.dma_start(xt[:], xT_view[:, :, t * 128:(t + 1) * 128])
                psF = fps.tile([128, WB_COLS], f32, tag="psF")
                for c in range(3):
                    nc.tensor.matmul(psF[:], xt[:, c, :], wbig_s[:, c, :],
                                     start=(c == 0), stop=(c == 2))
                pooled = fsb.tile([128, NGRP], f32, tag="pooled")
                nc.vector.tensor_reduce(
                    pooled[:], psF[:].rearrange("p (g w) -> p g w", w=GCOL),
                    axis=mybir.AxisListType.X, op=mybir.AluOpType.max)
                feats = fsb.tile([128, NGRP], f32, tag="feats")
                nc.vector.tensor_add(feats[:], pooled[:], b14_s[:])
                nc.vector.tensor_scalar_max(feats[:], feats[:], 0.0)
                psT = fps2.tile([NGRP, 128], f32, tag="psT")
                nc.tensor.transpose(psT[:], feats[:], id_s[:])
                nc.scalar.copy(featsT[0:NGRP, t * 128:(t + 1) * 128], psT[:])

        # ---- stage 2: qT, kT, V ----
        with tc.tile_pool(name="q_ps", bufs=2, space="PSUM") as qps, \
             tc.tile_pool(name="v_ps", bufs=2, space="PSUM") as vps:
            for c in range(NQ // 512):
                ps = qps.tile([HID, 512], f32, tag="psQ")
                nc.tensor.matmul(ps[:], wq_s[:],
                                 featsT[:, c * 512:(c + 1) * 512],
                                 start=True, stop=True)
                nc.scalar.copy(qTt[:, c * 512:(c + 1) * 512], ps[:])
            for c in range(N // 512):
                ps = qps.tile([HID, 512], f32, tag="psQ")
                nc.tensor.matmul(ps[:], wk_s[:],
                                 featsT[:, c * 512:(c + 1) * 512],
                                 start=True, stop=True)
                nc.scalar.copy(kTt[:, c * 512:(c + 1) * 512], ps[:])
            for c in range(KT):
                ps = vps.tile([128, HID + 1], f32, tag="psV")
                nc.tensor.matmul(ps[:], featsT[:, c * 128:(c + 1) * 128], wv_s[:],
                                 start=True, stop=True)
                nc.scalar.copy(vB[:, c, :], ps[:])

        # ---- stage 3: attention per query tile ----
        with tc.tile_pool(name="a_sb", bufs=2) as asb, \
             tc.tile_pool(name="a_small", bufs=2) as asm, \
             tc.tile_pool(name="s_ps", bufs=3, space="PSUM") as sps, \
             tc.tile_pool(name="ct_ps", bufs=2, space="PSUM") as ctps, \
             tc.tile_pool(name="c_ps", bufs=2, space="PSUM") as cps:
            for t in range(QT):
                qs = t * 128
                E = asb.tile([128, N], f32, tag="E")
                for c in range(N // 512):
                    psS = sps.tile([128, 512], f32, tag="psS")
                    nc.tensor.matmul(psS[:], qTt[:, qs:qs + 128],
                                     kTt[:, c * 512:(c + 1) * 512],
                                     start=True, stop=True)
                    nc.scalar.activation(E[:, c * 512:(c + 1) * 512], psS[:],
                                         mybir.ActivationFunctionType.Exp,
                                         scale=SCALE)

                # candidates: top-8 of each 128-key chunk
                candA = asm.tile([128, NCAND], f32, tag="candA")
                candB = asm.tile([128, NCAND], f32, tag="candB")
                for c in range(CH):
                    nc.vector.max(candA[:, c * 8:(c + 1) * 8],
                                  E[:, c * CHL:(c + 1) * CHL])

                # selection sort: 26 x (max8 + match_replace)
                srt = asm.tile([128, NSORT_IT * 8], f32, tag="srt")
                bufs = [candA, candB]
                for i in range(NSORT_IT):
                    src = bufs[i % 2]
                    dst = bufs[(i + 1) % 2]
                    nc.vector.max(srt[:, i * 8:(i + 1) * 8], src[:])
                    if i != NSORT_IT - 1:
                        nc.vector.match_replace(dst[:], srt[:, i * 8:(i + 1) * 8],
                                                src[:], 0.0)

                # probs = sorted / sum(sorted[:204])
                zs = asm.tile([128, 1], f32, tag="zs")
                rz = asm.tile([128, 1], f32, tag="rz")
                nc.vector.tensor_reduce(zs[:], srt[:, 0:K],
                                        axis=mybir.AxisListType.X,
                                        op=mybir.AluOpType.add)
                nc.vector.reciprocal(rz[:], zs[:])
                probs = asm.tile([128, K], f32, tag="probs")
                nc.scalar.activation(probs[:], srt[:, 0:K],
                                     mybir.ActivationFunctionType.Copy,
                                     scale=rz[:])
                nc.sync.dma_start(probs_out[qs:qs + 128, :], probs[:])

                # --- secant-Newton count-corrected mask threshold ---
                ALU = mybir.AluOpType
                AF = mybir.ActivationFunctionType
                Pbf = asb.tile([128, N], bf16, tag="Pbf")

                def count_ge(tau_ap, r_out):
                    # R = count(E >= tau) = (sum(sign(E - tau)) + 4097) / 2
                    ntau = asm.tile([128, 1], f32, tag="ntau")
                    nc.vector.tensor_scalar_mul(ntau[:], tau_ap, -1.0)
                    sgn = asm.tile([128, 1], f32, tag="sgn")
                    nc.scalar.activation(Pbf[:], E[:], AF.Sign,
                                         bias=ntau[:], accum_out=sgn[:])
                    nc.vector.tensor_scalar(r_out, sgn[:], 0.5, 2048.5,
                                            op0=ALU.mult, op1=ALU.add)

                def window_extract(j_ap, tau_out):
                    # tau = srt[j] (lerped for fractional/half-integer j)
                    jneg = asm.tile([128, 1], f32, tag="jneg")
                    nc.vector.tensor_scalar_mul(jneg[:], j_ap, -1.0)
                    w208 = asm.tile([128, NSORT_IT * 8], f32, tag="w208")
                    nc.scalar.activation(w208[:], iota_s[:, 0:NSORT_IT * 8],
                                         AF.Abs, bias=jneg[:])
                    nc.scalar.activation(w208[:], w208[:], AF.Relu,
                                         scale=-1.0, bias=1.0)
                    dummy = asm.tile([128, NSORT_IT * 8], f32, tag="dummy")
                    nc.vector.scalar_tensor_tensor(
                        dummy[:], w208[:], 0.0, srt[:],
                        op0=ALU.add, op1=ALU.mult, accum_out=tau_out)

                r0 = asm.tile([128, 1], f32, tag="r0")
                count_ge(srt[:, K - 1:K], r0[:])
                # j1 = clip(407 - R0, 100, 207)
                j1 = asm.tile([128, 1], f32, tag="j1")
                nc.vector.tensor_scalar(j1[:], r0[:], -1.0, 407.0,
                                        op0=ALU.mult, op1=ALU.add)
                nc.vector.tensor_scalar(j1[:], j1[:], 100.0, 207.0,
                                        op0=ALU.max, op1=ALU.min)
                tau1 = asm.tile([128, 1], f32, tag="tau1")
                window_extract(j1[:], tau1[:])
                r1 = asm.tile([128, 1], f32, tag="r1")
                count_ge(tau1[:], r1[:])
                # slope = max(R0 - R1, 1) / max(203 - j1, 1); j2 = j1 + (204-R1)/slope
                num = asm.tile([128, 1], f32, tag="num")
                nc.vector.tensor_scalar(num[:], j1[:], -1.0, 203.0,
                                        op0=ALU.mult, op1=ALU.add)
                nc.vector.tensor_scalar_max(num[:], num[:], 1.0)
                den = asm.tile([128, 1], f32, tag="den")
                nc.vector.tensor_tensor(den[:], r0[:], r1[:], op=ALU.subtract)
                nc.vector.tensor_scalar_max(den[:], den[:], 1.0)
                nc.vector.reciprocal(den[:], den[:])
                nc.vector.tensor_tensor(num[:], num[:], den[:], op=ALU.mult)
                nc.vector.tensor_scalar_min(num[:], num[:], 1.0)  # 1/slope
                step = asm.tile([128, 1], f32, tag="step")
                nc.vector.tensor_scalar(step[:], r1[:], -1.0, 204.0,
                                        op0=ALU.mult, op1=ALU.add)
                nc.vector.tensor_tensor(step[:], step[:], num[:], op=ALU.mult)
                j2 = asm.tile([128, 1], f32, tag="j2")
                nc.vector.tensor_tensor(j2[:], j1[:], step[:], op=ALU.add)
                nc.vector.tensor_scalar(j2[:], j2[:], 100.0, 207.0,
                                        op0=ALU.max, op1=ALU.min)
                tau2 = asm.tile([128, 1], f32, tag="tau2")
                window_extract(j2[:], tau2[:])

                # dense masked weights P = E * (E >= tau2), bf16
                nc.vector.scalar_tensor_tensor(
                    Pbf[:], E[:], tau2[:], E[:],
                    op0=ALU.is_ge, op1=ALU.mult)

                # PT[k, c, q] = P[q, c*128+k] via one xbar DMA transpose
                PT = asb.tile([128, KT, 128], bf16, tag="PT")
                nc.sync.dma_start(PT[:], Pbf[:], transpose=True)

                # ctxT[d, q] = sum_k V[k, d] * PT[k, q]
                psCt = ctps.tile([HID + 1, 128], f32, tag="psCt")
                for c in range(KT):
                    nc.tensor.matmul(psCt[:], vB[:, c, :], PT[:, c, :],
                                     start=(c == 0), stop=(c == KT - 1))
                ctxT = asm.tile([HID + 1, 128], f32, tag="ctxT")
                nc.scalar.copy(ctxT[:], psCt[:])
                psC2 = cps.tile([128, HID + 1], f32, tag="psC2")
                nc.tensor.transpose(psC2[:], ctxT[:], id_s[0:HID + 1, 0:HID + 1])
                rzd = asm.tile([128, 1], f32, tag="rzd")
                nc.vector.reciprocal(rzd[:], psC2[:, HID:HID + 1])
                ctxn = asm.tile([128, HID], f32, tag="ctxn")
                nc.scalar.activation(ctxn[:], psC2[:, 0:HID],
                                     mybir.ActivationFunctionType.Copy,
                                     scale=rzd[:])
                nc.vector.tensor_add(acc[:], acc[:], ctxn[:])

            nc.sync.dma_start(acc_out[:], acc[:])

    nc.compile()
    return nc


_NC_CACHE = None


def _get_nc():
    global _NC_CACHE
    if _NC_CACHE is None:
        _NC_CACHE = build_nc()
    return _NC_CACHE


def _host_inputs(inputs):
    """Build per-core input maps."""
    x = np.asarray(inputs["x"], np.float32)
    wbig = _build_wbig(inputs)

    def wxx15(w, b, extra_ones=False):
        o = w.shape[0]
        cols = o + (1 if extra_ones else 0)
        m = np.zeros((15, cols), np.float32)
        m[0:AFN, 0:o] = np.asarray(w, np.float32).T
        m[14, 0:o] = np.asarray(b, np.float32)
        if extra_ones:
            m[14, o] = 1.0
        return m

    wq15 = wxx15(inputs["w_q"], inputs["b_q"])
    wk15 = wxx15(inputs["w_k"], inputs["b_k"])
    wv15 = wxx15(inputs["w_v"], inputs["b_v"], extra_ones=True)
    bias14 = np.broadcast_to(
        np.concatenate([np.asarray(inputs["conv_b%d" % h], np.float32)
                        for h in KS])[None, :], (128, NGRP)).copy()
    ident = np.eye(128, dtype=np.float32)

    in_maps = []
    for core in range(8):
        b, h = core // 2, core % 2
        xb = x[b].reshape(N, XROWS)
        if h == 1:
            xb = np.concatenate([xb[NQ:], xb[:NQ]], axis=0)
        xT = np.empty((XR_PAD, N), np.float32)
        xT[0:XROWS] = xb.T
        xT[XROWS] = 1.0
        xT[XROWS + 1:] = 0.0
        in_maps.append({
            "xT": np.ascontiguousarray(xT),
            "wbig": wbig, "wq15": wq15, "wk15": wk15, "wv15": wv15,
            "bias14": bias14, "ident": ident,
            "ones_row": np.ones((1, N), np.float32),
            "iota_row": np.broadcast_to(
                np.arange(256, dtype=np.float32)[None, :], (128, 256)).copy(),
        })
    return in_maps


def kernel(**inputs):
    nc = _get_nc()
    in_maps = _host_inputs(inputs)
    res = run_bass_kernel_spmd(nc, in_maps, core_ids=list(range(8)))
    return _assemble(inputs, res.results)


def _assemble(inputs, results):
    probs = np.empty((B, N, K), np.float32)
    ctx_sum = np.zeros((B, HID), np.float64)
    for core in range(8):
        b, h = core // 2, core % 2
        r = results[core]
        probs[b, h * NQ:(h + 1) * NQ] = r["probs_out"]
        ctx_sum[b] += r["acc_out"].sum(axis=0, dtype=np.float64)

    w_attn = np.asarray(inputs["w_attn"], np.float64)
    b_attn = np.asarray(inputs["b_attn"], np.float64)
    w_mil = np.asarray(inputs["w_mil"], np.float64)
    b_mil = np.asarray(inputs["b_mil"], np.float64)
    wc = w_mil @ w_attn                     # [2, 10]
    bc = b_attn @ w_mil.T + b_mil           # [2]
    pooled = (ctx_sum @ wc.T) / N + bc
    return (pooled.astype(np.float32),
            probs.reshape(B, 1, N, 1, K).astype(np.float32))


# revision 12
# speedup vs baseline: 1.3282x; 1.0997x over previous
"""BiFormer sparse-attention kernel for 8 Trainium2 NeuronCores.

Layout: core c handles batch b=c//2, query half h=c%2 (2048 queries each).
Keys for each core are column-permuted so its own 2048 queries occupy
positions 0:2048 (key permutation is harmless: top-k/softmax/ctx are
permutation-invariant over keys).
"""

import math
from contextlib import ExitStack

import numpy as np

import concourse.bass as bass
import concourse.mybir as mybir
import concourse.tile as tile
from concourse import bacc
from concourse.bass_utils import run_bass_kernel_spmd

# ---- problem constants (hardcoded per task contract) ----
B, N, F, A = 4, 4096, 15, 24
HID, HD, AFN = 10, 10, 14
KS = [2, 3, 4, 5, 6, 7]
FN = [3, 3, 3, 2, 2, 1]
K = 204                       # int(N * 0.05)
SCALE = float(np.float32(1.0 / math.sqrt(HD)))
NQ = N // 2                   # queries per core
QT = NQ // 128                # query tiles per core (16)
CH = 32                       # key chunks for candidate extraction
CHL = N // CH                 # chunk length (128)
NCAND = CH * 8                # candidate count (256)
NSORT_IT = (K + 7) // 8       # 26 -> extracts 208 sorted values
XROWS = F * A                 # 360
XR_PAD = 363                  # 3 chunks of 121 partitions
GCOL = 24                     # padded positions per conv group
NGRP = AFN                    # 14 output channels
WB_COLS = NGRP * GCOL         # 336
NEG = -1.0e9

_dt = mybir.dt


def _build_wbig(inputs):
    """Block-Toeplitz conv weights [363, 336]; row 360 = const-1 row used to
    kill padded positions with NEG; rows 361/362 zero padding."""
    wb = np.zeros((XR_PAD, WB_COLS), np.float32)
    g = 0
    for h, f in zip(KS, FN):
        w = inputs["conv_w%d" % h]  # [f, F, h]
        npos = A - h + 1
        for o in range(f):
            base = g * GCOL
            for p in range(npos):
                for c in range(F):
                    for j in range(h):
                        wb[c * A + p + j, base + p] += w[o, c, j]
            wb[XROWS, base + npos:base + GCOL] = NEG
            g += 1
    assert g == NGRP
    return wb


def build_nc():
    nc = bacc.Bacc("TRN2", target_bir_lowering=False, debug=False,
                   enable_asserts=False, num_devices=8)
    f32 = _dt.float32

    xT_d = nc.dram_tensor("xT", [XR_PAD, N], f32, kind="ExternalInput").ap()
    wbig_d = nc.dram_tensor("wbig", [XR_PAD, WB_COLS], f32, kind="ExternalInput").ap()
    wq15_d = nc.dram_tensor("wq15", [15, HID], f32, kind="ExternalInput").ap()
    wk15_d = nc.dram_tensor("wk15", [15, HID], f32, kind="ExternalInput").ap()
    wv15_d = nc.dram_tensor("wv15", [15, HID + 1], f32, kind="ExternalInput").ap()
    bias14_d = nc.dram_tensor("bias14", [128, NGRP], f32, kind="ExternalInput").ap()
    ident_d = nc.dram_tensor("ident", [128, 128], f32, kind="ExternalInput").ap()
    ones_d = nc.dram_tensor("ones_row", [1, N], f32, kind="ExternalInput").ap()
    iota_d = nc.dram_tensor("iota_row", [128, 256], f32, kind="ExternalInput").ap()

    probs_out = nc.dram_tensor("probs_out", [NQ, K], f32, kind="ExternalOutput").ap()
    acc_out = nc.dram_tensor("acc_out", [128, HID], f32, kind="ExternalOutput").ap()

    KT = N // 128  # 32 key tiles
    f32r = _dt.float32r
    bf16 = _dt.bfloat16

    with tile.TileContext(nc) as tc, ExitStack() as ctx:
        # ---- long-lived SBUF ----
        persist = ctx.enter_context(tc.tile_pool(name="persist", bufs=1))
        featsT = persist.tile([15, N], f32, tag="featsT")
        qHi = persist.tile([HID, NQ], bf16, tag="qHi")
        qLo = persist.tile([HID, NQ], bf16, tag="qLo")
        kHi = persist.tile([HID, N], bf16, tag="kHi")
        kLo = persist.tile([HID, N], bf16, tag="kLo")
        vB = persist.tile([128, KT, HID + 1], bf16, tag="vB")
        iota_s = persist.tile([128, 256], f32, tag="iota")
        acc = persist.tile([128, HID], f32, tag="acc")
        wbig_s = persist.tile([121, 3, WB_COLS], f32, tag="wbig")
        wq_s = persist.tile([15, HID], f32, tag="wq")
        wk_s = persist.tile([15, HID], f32, tag="wk")
        wv_s = persist.tile([15, HID + 1], f32, tag="wv")
        b14_s = persist.tile([128, NGRP], f32, tag="b14")
        id_s = persist.tile([128, 128], f32, tag="id")

        nc.sync.dma_start(wbig_s[:], wbig_d.rearrange("(c p) f -> p c f", c=3))
        nc.sync.dma_start(wq_s[:], wq15_d[:])
        nc.sync.dma_start(wk_s[:], wk15_d[:])
        nc.sync.dma_start(wv_s[:], wv15_d[:])
        nc.sync.dma_start(b14_s[:], bias14_d[:])
        nc.sync.dma_start(id_s[:], ident_d[:])
        nc.sync.dma_start(iota_s[:], iota_d[:])
        nc.sync.dma_start(featsT[14:15, :], ones_d[:])
        nc.vector.memset(acc[:], 0.0)

        xT_view = xT_d.rearrange("(c p) n -> p c n", c=3)

        # ---- stage 1: features for all N keys ----
        with tc.tile_pool(name="f_sb", bufs=3) as fsb, \
             tc.tile_pool(name="f_ps", bufs=2, space="PSUM") as fps, \
             tc.tile_pool(name="f_ps2", bufs=2, space="PSUM") as fps2:
            for t in range(KT):
                xt = fsb.tile([121, 3, 128], f32, tag="xt")
                nc.sync.dma_start(xt[:], xT_view[:, :, t * 128:(t + 1) * 128])
                psF = fps.tile([128, WB_COLS], f32, tag="psF")
                for c in range(3):
                    nc.tensor.matmul(psF[:], xt[:, c, :], wbig_s[:, c, :],
                                     start=(c == 0), stop=(c == 2))
                pooled = fsb.tile([128, NGRP], f32, tag="pooled")
                nc.vector.tensor_reduce(
                    pooled[:], psF[:].rearrange("p (g w) -> p g w", w=GCOL),
                    axis=mybir.AxisListType.X, op=mybir.AluOpType.max)
                feats = fsb.tile([128, NGRP], f32, tag="feats")
                nc.vector.tensor_add(feats[:], pooled[:], b14_s[:])
                nc.vector.tensor_scalar_max(feats[:], feats[:], 0.0)
                psT = fps2.tile([NGRP, 128], f32, tag="psT")
                nc.tensor.transpose(psT[:], feats[:], id_s[:])
                nc.scalar.copy(featsT[0:NGRP, t * 128:(t + 1) * 128], psT[:])

        # ---- stage 2: qT, kT, V ----
        with tc.tile_pool(name="q_ps", bufs=2, space="PSUM") as qps, \
             tc.tile_pool(name="v_ps", bufs=2, space="PSUM") as vps:
            def split_hilo(ps_ap, hi_ap, lo_ap):
                # hi = bf16(x); lo = bf16(x - hi)
                nc.scalar.copy(hi_ap, ps_ap)
                nc.vector.tensor_tensor(lo_ap, ps_ap, hi_ap,
                                        op=mybir.AluOpType.subtract)

            for c in range(NQ // 512):
                ps = qps.tile([HID, 512], f32, tag="psQ")
                nc.tensor.matmul(ps[:], wq_s[:],
                                 featsT[:, c * 512:(c + 1) * 512],
                                 start=True, stop=True)
                split_hilo(ps[:], qHi[:, c * 512:(c + 1) * 512],
                           qLo[:, c * 512:(c + 1) * 512])
            for c in range(N // 512):
                ps = qps.tile([HID, 512], f32, tag="psQ")
                nc.tensor.matmul(ps[:], wk_s[:],
                                 featsT[:, c * 512:(c + 1) * 512],
                                 start=True, stop=True)
                split_hilo(ps[:], kHi[:, c * 512:(c + 1) * 512],
                           kLo[:, c * 512:(c + 1) * 512])
            for c in range(KT):
                ps = vps.tile([128, HID + 1], f32, tag="psV")
                nc.tensor.matmul(ps[:], featsT[:, c * 128:(c + 1) * 128], wv_s[:],
                                 start=True, stop=True)
                nc.scalar.copy(vB[:, c, :], ps[:])

        # ---- stage 3: attention per query tile ----
        with tc.tile_pool(name="a_sb", bufs=3) as asb, \
             tc.tile_pool(name="a_small", bufs=3) as asm, \
             tc.tile_pool(name="s_ps", bufs=3, space="PSUM") as sps, \
             tc.tile_pool(name="ct_ps", bufs=2, space="PSUM") as ctps, \
             tc.tile_pool(name="c_ps", bufs=2, space="PSUM") as cps:
            for t in range(QT):
                qs = t * 128
                E = asb.tile([128, N], f32, tag="E")
                for c in range(N // 512):
                    psS = sps.tile([128, 512], f32, tag="psS")
                    ksl = slice(c * 512, (c + 1) * 512)
                    nc.tensor.matmul(psS[:], qHi[:, qs:qs + 128], kHi[:, ksl],
                                     start=True, stop=False)
                    nc.tensor.matmul(psS[:], qHi[:, qs:qs + 128], kLo[:, ksl],
                                     start=False, stop=False)
                    nc.tensor.matmul(psS[:], qLo[:, qs:qs + 128], kHi[:, ksl],
                                     start=False, stop=True)
                    nc.scalar.activation(E[:, c * 512:(c + 1) * 512], psS[:],
                                         mybir.ActivationFunctionType.Exp,
                                         scale=SCALE)

                # candidates: top-8 of each 128-key chunk
                candA = asm.tile([128, NCAND], f32, tag="candA")
                candB = asm.tile([128, NCAND], f32, tag="candB")
                for c in range(CH):
                    nc.vector.max(candA[:, c * 8:(c + 1) * 8],
                                  E[:, c * CHL:(c + 1) * CHL])

                # selection sort: 26 x (max8 + match_replace)
                srt = asm.tile([128, NSORT_IT * 8], f32, tag="srt")
                bufs = [candA, candB]
                for i in range(NSORT_IT):
                    src = bufs[i % 2]
                    dst = bufs[(i + 1) % 2]
                    nc.vector.max(srt[:, i * 8:(i + 1) * 8], src[:])
                    if i != NSORT_IT - 1:
                        nc.vector.match_replace(dst[:], srt[:, i * 8:(i + 1) * 8],
                                                src[:], 0.0)

                # probs = sorted / sum(sorted[:204])
                zs = asm.tile([128, 1], f32, tag="zs")
                rz = asm.tile([128, 1], f32, tag="rz")
                nc.vector.tensor_reduce(zs[:], srt[:, 0:K],
                                        axis=mybir.AxisListType.X,
                                        op=mybir.AluOpType.add)
                nc.vector.reciprocal(rz[:], zs[:])
                probs = asm.tile([128, K], f32, tag="probs")
                nc.scalar.activation(probs[:], srt[:, 0:K],
                                     mybir.ActivationFunctionType.Copy,
                                     scale=rz[:])
                nc.sync.dma_start(probs_out[qs:qs + 128, :], probs[:])

                # --- secant-Newton count-corrected mask threshold ---
                ALU = mybir.AluOpType
                AF = mybir.ActivationFunctionType
                Pbf = asb.tile([128, N], bf16, tag="Pbf")

                def count_ge(tau_ap, r_out):
                    # R = count(E >= tau) = (sum(sign(E - tau)) + 4097) / 2
                    ntau = asm.tile([128, 1], f32, tag="ntau")
                    nc.vector.tensor_scalar_mul(ntau[:], tau_ap, -1.0)
                    sgn = asm.tile([128, 1], f32, tag="sgn")
                    nc.scalar.activation(Pbf[:], E[:], AF.Sign,
                                         bias=ntau[:], accum_out=sgn[:])
                    nc.vector.tensor_scalar(r_out, sgn[:], 0.5, 2048.5,
                                            op0=ALU.mult, op1=ALU.add)

                def window_extract(j_ap, tau_out):
                    # tau = srt[j] (lerped for fractional/half-integer j)
                    jneg = asm.tile([128, 1], f32, tag="jneg")
                    nc.vector.tensor_scalar_mul(jneg[:], j_ap, -1.0)
                    w208 = asm.tile([128, NSORT_IT * 8], f32, tag="w208")
                    nc.scalar.activation(w208[:], iota_s[:, 0:NSORT_IT * 8],
                                         AF.Abs, bias=jneg[:])
                    nc.scalar.activation(w208[:], w208[:], AF.Relu,
                                         scale=-1.0, bias=1.0)
                    dummy = asm.tile([128, NSORT_IT * 8], f32, tag="dummy")
                    nc.vector.scalar_tensor_tensor(
                        dummy[:], w208[:], 0.0, srt[:],
                        op0=ALU.add, op1=ALU.mult, accum_out=tau_out)

                r0 = asm.tile([128, 1], f32, tag="r0")
                count_ge(srt[:, K - 1:K], r0[:])
                # j1 = clip(408 - R0, 100, 207)   (target: count_ge = 205)
                j1 = asm.tile([128, 1], f32, tag="j1")
                nc.vector.tensor_scalar(j1[:], r0[:], -1.0, 408.0,
                                        op0=ALU.mult, op1=ALU.add)
                nc.vector.tensor_scalar(j1[:], j1[:], 100.0, 207.0,
                                        op0=ALU.max, op1=ALU.min)
                tau1 = asm.tile([128, 1], f32, tag="tau1")
                window_extract(j1[:], tau1[:])
                r1 = asm.tile([128, 1], f32, tag="r1")
                count_ge(tau1[:], r1[:])
                # slope = max(R0 - R1, 1) / max(203 - j1, 1); j2 = j1 + (204-R1)/slope
                num = asm.tile([128, 1], f32, tag="num")
                nc.vector.tensor_scalar(num[:], j1[:], -1.0, 203.0,
                                        op0=ALU.mult, op1=ALU.add)
                nc.vector.tensor_scalar_max(num[:], num[:], 1.0)
                den = asm.tile([128, 1], f32, tag="den")
                nc.vector.tensor_tensor(den[:], r0[:], r1[:], op=ALU.subtract)
                nc.vector.tensor_scalar_max(den[:], den[:], 1.0)
                nc.vector.reciprocal(den[:], den[:])
                nc.vector.tensor_tensor(num[:], num[:], den[:], op=ALU.mult)
                nc.vector.tensor_scalar_min(num[:], num[:], 1.0)  # 1/slope
                step = asm.tile([128, 1], f32, tag="step")
                nc.vector.tensor_scalar(step[:], r1[:], -1.0, 205.0,
                                        op0=ALU.mult, op1=ALU.add)
                nc.vector.tensor_tensor(step[:], step[:], num[:], op=ALU.mult)
                j2 = asm.tile([128, 1], f32, tag="j2")
                nc.vector.tensor_tensor(j2[:], j1[:], step[:], op=ALU.add)
                nc.vector.tensor_scalar(j2[:], j2[:], 100.0, 207.0,
                                        op0=ALU.max, op1=ALU.min)
                tau2 = asm.tile([128, 1], f32, tag="tau2")
                window_extract(j2[:], tau2[:])

                # dense masked weights P = E * (E > tau2):
                # m = relu(sign(E - tau2)) [ACT+DVE], P = E * m [Pool]
                ntau2 = asm.tile([128, 1], f32, tag="ntau2")
                nc.vector.tensor_scalar_mul(ntau2[:], tau2[:], -1.0)
                mbf = asb.tile([128, N], bf16, tag="mbf")
                nc.scalar.activation(mbf[:], E[:], AF.Sign, bias=ntau2[:])
                nc.vector.tensor_scalar_max(mbf[:], mbf[:], 0.0)
                nc.gpsimd.tensor_tensor(Pbf[:], E[:], mbf[:], op=ALU.mult)

                # PT[k, c, q] = P[q, c*128+k] via one xbar DMA transpose
                PT = asb.tile([128, KT, 128], bf16, tag="PT")
                nc.sync.dma_start(PT[:], Pbf[:], transpose=True)

                # ctxT[d, q] = sum_k V[k, d] * PT[k, q]
                psCt = ctps.tile([HID + 1, 128], f32, tag="psCt")
                for c in range(KT):
                    nc.tensor.matmul(psCt[:], vB[:, c, :], PT[:, c, :],
                                     start=(c == 0), stop=(c == KT - 1))
                ctxT = asm.tile([HID + 1, 128], f32, tag="ctxT")
                nc.scalar.copy(ctxT[:], psCt[:])
                psC2 = cps.tile([128, HID + 1], f32, tag="psC2")
                nc.tensor.transpose(psC2[:], ctxT[:], id_s[0:HID + 1, 0:HID + 1])
                rzd = asm.tile([128, 1], f32, tag="rzd")
                nc.vector.reciprocal(rzd[:], psC2[:, HID:HID + 1])
                ctxn = asm.tile([128, HID], f32, tag="ctxn")
                nc.scalar.activation(ctxn[:], psC2[:, 0:HID],
                                     mybir.ActivationFunctionType.Copy,
                                     scale=rzd[:])
                nc.vector.tensor_add(acc[:], acc[:], ctxn[:])

            nc.sync.dma_start(acc_out[:], acc[:])

    nc.compile()
    return nc


_NC_CACHE = None


def _get_nc():
    global _NC_CACHE
    if _NC_CACHE is None:
        _NC_CACHE = build_nc()
    return _NC_CACHE


def _host_inputs(inputs):
    """Build per-core input maps."""
    x = np.asarray(inputs["x"], np.float32)
    wbig = _build_wbig(inputs)

    def wxx15(w, b, extra_ones=False):
        o = w.shape[0]
        cols = o + (1 if extra_ones else 0)
        m = np.zeros((15, cols), np.float32)
        m[0:AFN, 0:o] = np.asarray(w, np.float32).T
        m[14, 0:o] = np.asarray(b, np.float32)
        if extra_ones:
            m[14, o] = 1.0
        return m

    wq15 = wxx15(inputs["w_q"], inputs["b_q"])
    wk15 = wxx15(inputs["w_k"], inputs["b_k"])
    wv15 = wxx15(inputs["w_v"], inputs["b_v"], extra_ones=True)
    bias14 = np.broadcast_to(
        np.concatenate([np.asarray(inputs["conv_b%d" % h], np.float32)
                        for h in KS])[None, :], (128, NGRP)).copy()
    ident = np.eye(128, dtype=np.float32)

    in_maps = []
    for core in range(8):
        b, h = core // 2, core % 2
        xb = x[b].reshape(N, XROWS)
        if h == 1:
            xb = np.concatenate([xb[NQ:], xb[:NQ]], axis=0)
        xT = np.empty((XR_PAD, N), np.float32)
        xT[0:XROWS] = xb.T
        xT[XROWS] = 1.0
        xT[XROWS + 1:] = 0.0
        in_maps.append({
            "xT": np.ascontiguousarray(xT),
            "wbig": wbig, "wq15": wq15, "wk15": wk15, "wv15": wv15,
            "bias14": bias14, "ident": ident,
            "ones_row": np.ones((1, N), np.float32),
            "iota_row": np.broadcast_to(
                np.arange(256, dtype=np.float32)[None, :], (128, 256)).copy(),
        })
    return in_maps


def kernel(**inputs):
    nc = _get_nc()
    in_maps = _host_inputs(inputs)
    res = run_bass_kernel_spmd(nc, in_maps, core_ids=list(range(8)))
    return _assemble(inputs, res.results)


def _assemble(inputs, results):
    probs = np.empty((B, N, K), np.float32)
    ctx_sum = np.zeros((B, HID), np.float64)
    for core in range(8):
        b, h = core // 2, core % 2
        r = results[core]
        probs[b, h * NQ:(h + 1) * NQ] = r["probs_out"]
        ctx_sum[b] += r["acc_out"].sum(axis=0, dtype=np.float64)

    w_attn = np.asarray(inputs["w_attn"], np.float64)
    b_attn = np.asarray(inputs["b_attn"], np.float64)
    w_mil = np.asarray(inputs["w_mil"], np.float64)
    b_mil = np.asarray(inputs["b_mil"], np.float64)
    wc = w_mil @ w_attn                     # [2, 10]
    bc = b_attn @ w_mil.T + b_mil           # [2]
    pooled = (ctx_sum @ wc.T) / N + bc
    return (pooled.astype(np.float32),
            probs.reshape(B, 1, N, 1, K).astype(np.float32))


# revision 13
# speedup vs baseline: 1.3381x; 1.0074x over previous
"""BiFormer sparse-attention kernel for 8 Trainium2 NeuronCores.

Layout: core c handles batch b=c//2, query half h=c%2 (2048 queries each).
Keys for each core are column-permuted so its own 2048 queries occupy
positions 0:2048 (key permutation is harmless: top-k/softmax/ctx are
permutation-invariant over keys).
"""

import math
from contextlib import ExitStack

import numpy as np

import concourse.bass as bass
import concourse.mybir as mybir
import concourse.tile as tile
from concourse import bacc
from concourse.bass_utils import run_bass_kernel_spmd

# ---- problem constants (hardcoded per task contract) ----
B, N, F, A = 4, 4096, 15, 24
HID, HD, AFN = 10, 10, 14
KS = [2, 3, 4, 5, 6, 7]
FN = [3, 3, 3, 2, 2, 1]
K = 204                       # int(N * 0.05)
SCALE = float(np.float32(1.0 / math.sqrt(HD)))
NQ = N // 2                   # queries per core
QT = NQ // 128                # query tiles per core (16)
CH = 32                       # key chunks for candidate extraction
CHL = N // CH                 # chunk length (128)
NCAND = CH * 8                # candidate count (256)
NSORT_IT = (K + 7) // 8       # 26 -> extracts 208 sorted values
XROWS = F * A                 # 360
XR_PAD = 363                  # 3 chunks of 121 partitions
GCOL = 24                     # padded positions per conv group
NGRP = AFN                    # 14 output channels
WB_COLS = NGRP * GCOL         # 336
NEG = -1.0e9

_dt = mybir.dt


def _build_wbig(inputs):
    """Block-Toeplitz conv weights [363, 336]; row 360 = const-1 row used to
    kill padded positions with NEG; rows 361/362 zero padding."""
    wb = np.zeros((XR_PAD, WB_COLS), np.float32)
    g = 0
    for h, f in zip(KS, FN):
        w = inputs["conv_w%d" % h]  # [f, F, h]
        npos = A - h + 1
        for o in range(f):
            base = g * GCOL
            for p in range(npos):
                for c in range(F):
                    for j in range(h):
                        wb[c * A + p + j, base + p] += w[o, c, j]
            wb[XROWS, base + npos:base + GCOL] = NEG
            g += 1
    assert g == NGRP
    return wb


def build_nc():
    nc = bacc.Bacc("TRN2", target_bir_lowering=False, debug=False,
                   enable_asserts=False, num_devices=8)
    f32 = _dt.float32

    xT_d = nc.dram_tensor("xT", [XR_PAD, N], f32, kind="ExternalInput").ap()
    wbig_d = nc.dram_tensor("wbig", [XR_PAD, WB_COLS], f32, kind="ExternalInput").ap()
    wq15_d = nc.dram_tensor("wq15", [15, HID], f32, kind="ExternalInput").ap()
    wk15_d = nc.dram_tensor("wk15", [15, HID], f32, kind="ExternalInput").ap()
    wv15_d = nc.dram_tensor("wv15", [15, HID + 1], f32, kind="ExternalInput").ap()
    bias14_d = nc.dram_tensor("bias14", [128, NGRP], f32, kind="ExternalInput").ap()
    ident_d = nc.dram_tensor("ident", [128, 128], f32, kind="ExternalInput").ap()
    ones_d = nc.dram_tensor("ones_row", [1, N], f32, kind="ExternalInput").ap()
    iota_d = nc.dram_tensor("iota_row", [128, 256], f32, kind="ExternalInput").ap()

    probs_out = nc.dram_tensor("probs_out", [NQ, K], f32, kind="ExternalOutput").ap()
    acc_out = nc.dram_tensor("acc_out", [128, HID], f32, kind="ExternalOutput").ap()

    KT = N // 128  # 32 key tiles
    f32r = _dt.float32r
    bf16 = _dt.bfloat16

    with tile.TileContext(nc) as tc, ExitStack() as ctx:
        # ---- long-lived SBUF ----
        persist = ctx.enter_context(tc.tile_pool(name="persist", bufs=1))
        featsT = persist.tile([15, N], f32, tag="featsT")
        qHi = persist.tile([HID, NQ], bf16, tag="qHi")
        qLo = persist.tile([HID, NQ], bf16, tag="qLo")
        kHi = persist.tile([HID, N], bf16, tag="kHi")
        kLo = persist.tile([HID, N], bf16, tag="kLo")
        vB = persist.tile([128, KT, HID + 1], bf16, tag="vB")
        iota_s = persist.tile([128, 256], f32, tag="iota")
        acc = persist.tile([128, HID], f32, tag="acc")
        wbig_s = persist.tile([121, 3, WB_COLS], f32, tag="wbig")
        wq_s = persist.tile([15, HID], f32, tag="wq")
        wk_s = persist.tile([15, HID], f32, tag="wk")
        wv_s = persist.tile([15, HID + 1], f32, tag="wv")
        b14_s = persist.tile([128, NGRP], f32, tag="b14")
        id_s = persist.tile([128, 128], f32, tag="id")

        nc.sync.dma_start(wbig_s[:], wbig_d.rearrange("(c p) f -> p c f", c=3))
        nc.sync.dma_start(wq_s[:], wq15_d[:])
        nc.sync.dma_start(wk_s[:], wk15_d[:])
        nc.sync.dma_start(wv_s[:], wv15_d[:])
        nc.sync.dma_start(b14_s[:], bias14_d[:])
        nc.sync.dma_start(id_s[:], ident_d[:])
        nc.sync.dma_start(iota_s[:], iota_d[:])
        nc.sync.dma_start(featsT[14:15, :], ones_d[:])
        nc.vector.memset(acc[:], 0.0)

        xT_view = xT_d.rearrange("(c p) n -> p c n", c=3)

        # ---- stage 1: features for all N keys ----
        with tc.tile_pool(name="f_sb", bufs=3) as fsb, \
             tc.tile_pool(name="f_ps", bufs=2, space="PSUM") as fps, \
             tc.tile_pool(name="f_ps2", bufs=2, space="PSUM") as fps2:
            for t in range(KT):
                xt = fsb.tile([121, 3, 128], f32, tag="xt")
                nc.sync.dma_start(xt[:], xT_view[:, :, t * 128:(t + 1) * 128])
                psF = fps.tile([128, WB_COLS], f32, tag="psF")
                for c in range(3):
                    nc.tensor.matmul(psF[:], xt[:, c, :], wbig_s[:, c, :],
                                     start=(c == 0), stop=(c == 2))
                pooled = fsb.tile([128, NGRP], f32, tag="pooled")
                nc.vector.tensor_reduce(
                    pooled[:], psF[:].rearrange("p (g w) -> p g w", w=GCOL),
                    axis=mybir.AxisListType.X, op=mybir.AluOpType.max)
                feats = fsb.tile([128, NGRP], f32, tag="feats")
                nc.vector.tensor_add(feats[:], pooled[:], b14_s[:])
                nc.vector.tensor_scalar_max(feats[:], feats[:], 0.0)
                psT = fps2.tile([NGRP, 128], f32, tag="psT")
                nc.tensor.transpose(psT[:], feats[:], id_s[:])
                nc.scalar.copy(featsT[0:NGRP, t * 128:(t + 1) * 128], psT[:])

        # ---- stage 2: qT, kT, V ----
        with tc.tile_pool(name="q_ps", bufs=2, space="PSUM") as qps, \
             tc.tile_pool(name="v_ps", bufs=2, space="PSUM") as vps:
            def split_hilo(ps_ap, hi_ap, lo_ap):
                # hi = bf16(x); lo = bf16(x - hi)
                nc.scalar.copy(hi_ap, ps_ap)
                nc.vector.tensor_tensor(lo_ap, ps_ap, hi_ap,
                                        op=mybir.AluOpType.subtract)

            for c in range(NQ // 512):
                ps = qps.tile([HID, 512], f32, tag="psQ")
                nc.tensor.matmul(ps[:], wq_s[:],
                                 featsT[:, c * 512:(c + 1) * 512],
                                 start=True, stop=True)
                split_hilo(ps[:], qHi[:, c * 512:(c + 1) * 512],
                           qLo[:, c * 512:(c + 1) * 512])
            for c in range(N // 512):
                ps = qps.tile([HID, 512], f32, tag="psQ")
                nc.tensor.matmul(ps[:], wk_s[:],
                                 featsT[:, c * 512:(c + 1) * 512],
                                 start=True, stop=True)
                split_hilo(ps[:], kHi[:, c * 512:(c + 1) * 512],
                           kLo[:, c * 512:(c + 1) * 512])
            for c in range(KT):
                ps = vps.tile([128, HID + 1], f32, tag="psV")
                nc.tensor.matmul(ps[:], featsT[:, c * 128:(c + 1) * 128], wv_s[:],
                                 start=True, stop=True)
                nc.scalar.copy(vB[:, c, :], ps[:])

        # ---- stage 3: attention per query tile ----
        with tc.tile_pool(name="a_sb", bufs=3) as asb, \
             tc.tile_pool(name="a_small", bufs=3) as asm, \
             tc.tile_pool(name="s_ps", bufs=3, space="PSUM") as sps, \
             tc.tile_pool(name="ct_ps", bufs=2, space="PSUM") as ctps, \
             tc.tile_pool(name="c_ps", bufs=2, space="PSUM") as cps:
            for t in range(QT):
                qs = t * 128
                E = asb.tile([128, N], f32, tag="E")
                for c in range(N // 512):
                    psS = sps.tile([128, 512], f32, tag="psS")
                    ksl = slice(c * 512, (c + 1) * 512)
                    nc.tensor.matmul(psS[:], qHi[:, qs:qs + 128], kHi[:, ksl],
                                     start=True, stop=False)
                    nc.tensor.matmul(psS[:], qHi[:, qs:qs + 128], kLo[:, ksl],
                                     start=False, stop=False)
                    nc.tensor.matmul(psS[:], qLo[:, qs:qs + 128], kHi[:, ksl],
                                     start=False, stop=True)
                    nc.scalar.activation(E[:, c * 512:(c + 1) * 512], psS[:],
                                         mybir.ActivationFunctionType.Exp,
                                         scale=SCALE)

                # candidates: top-8 of each 128-key chunk
                candA = asm.tile([128, NCAND], f32, tag="candA")
                candB = asm.tile([128, NCAND], f32, tag="candB")
                for c in range(CH):
                    nc.vector.max(candA[:, c * 8:(c + 1) * 8],
                                  E[:, c * CHL:(c + 1) * CHL])

                # selection sort: 26 x (max8 + match_replace)
                srt = asm.tile([128, NSORT_IT * 8], f32, tag="srt")
                bufs = [candA, candB]
                for i in range(NSORT_IT):
                    src = bufs[i % 2]
                    dst = bufs[(i + 1) % 2]
                    nc.vector.max(srt[:, i * 8:(i + 1) * 8], src[:])
                    if i != NSORT_IT - 1:
                        nc.vector.match_replace(dst[:], srt[:, i * 8:(i + 1) * 8],
                                                src[:], 0.0)

                # probs = sorted / sum(sorted[:204])
                zs = asm.tile([128, 1], f32, tag="zs")
                rz = asm.tile([128, 1], f32, tag="rz")
                nc.vector.tensor_reduce(zs[:], srt[:, 0:K],
                                        axis=mybir.AxisListType.X,
                                        op=mybir.AluOpType.add)
                nc.vector.reciprocal(rz[:], zs[:])
                probs = asm.tile([128, K], f32, tag="probs")
                nc.scalar.activation(probs[:], srt[:, 0:K],
                                     mybir.ActivationFunctionType.Copy,
                                     scale=rz[:])
                nc.sync.dma_start(probs_out[qs:qs + 128, :], probs[:])

                # --- secant-Newton count-corrected mask threshold ---
                ALU = mybir.AluOpType
                AF = mybir.ActivationFunctionType
                Pbf = asb.tile([128, N], bf16, tag="Pbf")

                def count_ge(tau_ap, r_out):
                    # R = count(E >= tau) = (sum(sign(E - tau)) + 4097) / 2
                    ntau = asm.tile([128, 1], f32, tag="ntau")
                    nc.vector.tensor_scalar_mul(ntau[:], tau_ap, -1.0)
                    sgn = asm.tile([128, 1], f32, tag="sgn")
                    nc.scalar.activation(Pbf[:], E[:], AF.Sign,
                                         bias=ntau[:], accum_out=sgn[:])
                    nc.vector.tensor_scalar(r_out, sgn[:], 0.5, 2048.5,
                                            op0=ALU.mult, op1=ALU.add)

                def window_extract(j_ap, tau_out):
                    # tau = srt[j] (lerped for fractional/half-integer j)
                    jneg = asm.tile([128, 1], f32, tag="jneg")
                    nc.vector.tensor_scalar_mul(jneg[:], j_ap, -1.0)
                    w208 = asm.tile([128, NSORT_IT * 8], f32, tag="w208")
                    nc.scalar.activation(w208[:], iota_s[:, 0:NSORT_IT * 8],
                                         AF.Abs, bias=jneg[:])
                    nc.scalar.activation(w208[:], w208[:], AF.Relu,
                                         scale=-1.0, bias=1.0)
                    dummy = asm.tile([128, NSORT_IT * 8], f32, tag="dummy")
                    nc.vector.scalar_tensor_tensor(
                        dummy[:], w208[:], 0.0, srt[:],
                        op0=ALU.add, op1=ALU.mult, accum_out=tau_out)

                # two-probe secant: R0 at srt[203], Rb at srt[185]
                JP = 185
                r0 = asm.tile([128, 1], f32, tag="r0")
                count_ge(srt[:, K - 1:K], r0[:])
                rb = asm.tile([128, 1], f32, tag="rb")
                count_ge(srt[:, JP:JP + 1], rb[:])
                den = asm.tile([128, 1], f32, tag="den")
                nc.vector.tensor_tensor(den[:], r0[:], rb[:], op=ALU.subtract)
                nc.vector.tensor_scalar_max(den[:], den[:], float(203 - JP))
                nc.vector.reciprocal(den[:], den[:])
                step = asm.tile([128, 1], f32, tag="step")
                nc.vector.tensor_scalar(step[:], r0[:], -205.0, float(203 - JP),
                                        op0=ALU.add, op1=ALU.mult)
                nc.vector.tensor_tensor(step[:], step[:], den[:], op=ALU.mult)
                j2 = asm.tile([128, 1], f32, tag="j2")
                nc.vector.tensor_scalar(step[:], step[:], -1.0, 203.0,
                                        op0=ALU.mult, op1=ALU.add)
                nc.vector.tensor_scalar(j2[:], step[:], 100.0, 207.0,
                                        op0=ALU.max, op1=ALU.min)
                tau2 = asm.tile([128, 1], f32, tag="tau2")
                window_extract(j2[:], tau2[:])
                ntau2 = asm.tile([128, 1], f32, tag="ntau2")
                nc.vector.tensor_scalar_mul(ntau2[:], tau2[:], -1.0)

                # blockwise: mask (ACT+DVE+Pool) -> xbar transpose -> ctx MM
                NB = 4
                BW = N // NB          # 1024 columns per block
                BC = KT // NB         # 8 key chunks per block
                mbf = asb.tile([128, N], bf16, tag="mbf")
                PT = asb.tile([128, KT, 128], bf16, tag="PT")
                psCt = ctps.tile([HID + 1, 128], f32, tag="psCt")
                for blk in range(NB):
                    csl = slice(blk * BW, (blk + 1) * BW)
                    nc.scalar.activation(mbf[:, csl], E[:, csl], AF.Sign,
                                         bias=ntau2[:])
                    nc.vector.tensor_scalar_max(mbf[:, csl], mbf[:, csl], 0.0)
                    nc.gpsimd.tensor_tensor(Pbf[:, csl], E[:, csl],
                                            mbf[:, csl], op=ALU.mult)
                    nc.sync.dma_start(PT[:, blk * BC:(blk + 1) * BC, :],
                                      Pbf[:, csl], transpose=True)
                    for cc in range(BC):
                        c = blk * BC + cc
                        nc.tensor.matmul(psCt[:], vB[:, c, :], PT[:, c, :],
                                         start=(c == 0), stop=(c == KT - 1))
                ctxT = asm.tile([HID + 1, 128], f32, tag="ctxT")
                nc.scalar.copy(ctxT[:], psCt[:])
                psC2 = cps.tile([128, HID + 1], f32, tag="psC2")
                nc.tensor.transpose(psC2[:], ctxT[:], id_s[0:HID + 1, 0:HID + 1])
                rzd = asm.tile([128, 1], f32, tag="rzd")
                nc.vector.reciprocal(rzd[:], psC2[:, HID:HID + 1])
                ctxn = asm.tile([128, HID], f32, tag="ctxn")
                nc.scalar.activation(ctxn[:], psC2[:, 0:HID],
                                     mybir.ActivationFunctionType.Copy,
                                     scale=rzd[:])
                nc.vector.tensor_add(acc[:], acc[:], ctxn[:])

            nc.sync.dma_start(acc_out[:], acc[:])

    nc.compile()
    return nc


_NC_CACHE = None


def _get_nc():
    global _NC_CACHE
    if _NC_CACHE is None:
        _NC_CACHE = build_nc()
    return _NC_CACHE


def _host_inputs(inputs):
    """Build per-core input maps."""
    x = np.asarray(inputs["x"], np.float32)
    wbig = _build_wbig(inputs)

    def wxx15(w, b, extra_ones=False):
        o = w.shape[0]
        cols = o + (1 if extra_ones else 0)
        m = np.zeros((15, cols), np.float32)
        m[0:AFN, 0:o] = np.asarray(w, np.float32).T
        m[14, 0:o] = np.asarray(b, np.float32)
        if extra_ones:
            m[14, o] = 1.0
        return m

    wq15 = wxx15(inputs["w_q"], inputs["b_q"])
    wk15 = wxx15(inputs["w_k"], inputs["b_k"])
    wv15 = wxx15(inputs["w_v"], inputs["b_v"], extra_ones=True)
    bias14 = np.broadcast_to(
        np.concatenate([np.asarray(inputs["conv_b%d" % h], np.float32)
                        for h in KS])[None, :], (128, NGRP)).copy()
    ident = np.eye(128, dtype=np.float32)

    in_maps = []
    for core in range(8):
        b, h = core // 2, core % 2
        xb = x[b].reshape(N, XROWS)
        if h == 1:
            xb = np.concatenate([xb[NQ:], xb[:NQ]], axis=0)
        xT = np.empty((XR_PAD, N), np.float32)
        xT[0:XROWS] = xb.T
        xT[XROWS] = 1.0
        xT[XROWS + 1:] = 0.0
        in_maps.append({
            "xT": np.ascontiguousarray(xT),
            "wbig": wbig, "wq15": wq15, "wk15": wk15, "wv15": wv15,
            "bias14": bias14, "ident": ident,
            "ones_row": np.ones((1, N), np.float32),
            "iota_row": np.broadcast_to(
                np.arange(256, dtype=np.float32)[None, :], (128, 256)).copy(),
        })
    return in_maps


def kernel(**inputs):
    nc = _get_nc()
    in_maps = _host_inputs(inputs)
    res = run_bass_kernel_spmd(nc, in_maps, core_ids=list(range(8)))
    return _assemble(inputs, res.results)


def _assemble(inputs, results):
    probs = np.empty((B, N, K), np.float32)
    ctx_sum = np.zeros((B, HID), np.float64)
    for core in range(8):
        b, h = core // 2, core % 2
        r = results[core]
        probs[b, h * NQ:(h + 1) * NQ] = r["probs_out"]
        ctx_sum[b] += r["acc_out"].sum(axis=0, dtype=np.float64)

    w_attn = np.asarray(inputs["w_attn"], np.float64)
    b_attn = np.asarray(inputs["b_attn"], np.float64)
    w_mil = np.asarray(inputs["w_mil"], np.float64)
    b_mil = np.asarray(inputs["b_mil"], np.float64)
    wc = w_mil @ w_attn                     # [2, 10]
    bc = b_attn @ w_mil.T + b_mil           # [2]
    pooled = (ctx_sum @ wc.T) / N + bc
    return (pooled.astype(np.float32),
            probs.reshape(B, 1, N, 1, K).astype(np.float32))


# revision 14
# speedup vs baseline: 1.5338x; 1.1463x over previous
"""BiFormer sparse-attention kernel for 8 Trainium2 NeuronCores.

Layout: core c handles batch b=c//2, query half h=c%2 (2048 queries each).
Keys for each core are column-permuted so its own 2048 queries occupy
positions 0:2048 (key permutation is harmless: top-k/softmax/ctx are
permutation-invariant over keys).
"""

import math
from contextlib import ExitStack

import numpy as np

import concourse.bass as bass
import concourse.mybir as mybir
import concourse.tile as tile
from concourse import bacc
from concourse.bass_utils import run_bass_kernel_spmd

# ---- problem constants (hardcoded per task contract) ----
B, N, F, A = 4, 4096, 15, 24
HID, HD, AFN = 10, 10, 14
KS = [2, 3, 4, 5, 6, 7]
FN = [3, 3, 3, 2, 2, 1]
K = 204                       # int(N * 0.05)
SCALE = float(np.float32(1.0 / math.sqrt(HD)))
NQ = N // 2                   # queries per core
QT = NQ // 128                # query tiles per core (16)
CH = 32                       # key chunks for candidate extraction
CHL = N // CH                 # chunk length (128)
NCAND = CH * 8                # candidate count (256)
NSORT_IT = (K + 7) // 8       # 26 -> extracts 208 sorted values
XROWS = F * A                 # 360
XR_PAD = 363                  # 3 chunks of 121 partitions
GCOL = 24                     # padded positions per conv group
NGRP = AFN                    # 14 output channels
WB_COLS = NGRP * GCOL         # 336
NEG = -1.0e9

_dt = mybir.dt


def _build_wbig(inputs):
    """Block-Toeplitz conv weights [363, 336]; row 360 = const-1 row used to
    kill padded positions with NEG; rows 361/362 zero padding."""
    wb = np.zeros((XR_PAD, WB_COLS), np.float32)
    g = 0
    for h, f in zip(KS, FN):
        w = inputs["conv_w%d" % h]  # [f, F, h]
        npos = A - h + 1
        for o in range(f):
            base = g * GCOL
            for p in range(npos):
                for c in range(F):
                    for j in range(h):
                        wb[c * A + p + j, base + p] += w[o, c, j]
            wb[XROWS, base + npos:base + GCOL] = NEG
            g += 1
    assert g == NGRP
    return wb


def build_nc():
    nc = bacc.Bacc("TRN2", target_bir_lowering=False, debug=False,
                   enable_asserts=False, num_devices=8)
    f32 = _dt.float32

    xT_d = nc.dram_tensor("xT", [XR_PAD, N], f32, kind="ExternalInput").ap()
    wbig_d = nc.dram_tensor("wbig", [XR_PAD, WB_COLS], f32, kind="ExternalInput").ap()
    wq15_d = nc.dram_tensor("wq15", [15, HID], f32, kind="ExternalInput").ap()
    wk15_d = nc.dram_tensor("wk15", [15, HID], f32, kind="ExternalInput").ap()
    wv15_d = nc.dram_tensor("wv15", [15, HID + 1], f32, kind="ExternalInput").ap()
    bias14_d = nc.dram_tensor("bias14", [128, NGRP], f32, kind="ExternalInput").ap()
    ident_d = nc.dram_tensor("ident", [128, 128], f32, kind="ExternalInput").ap()
    ones_d = nc.dram_tensor("ones_row", [1, N], f32, kind="ExternalInput").ap()
    iota_d = nc.dram_tensor("iota_row", [128, 256], f32, kind="ExternalInput").ap()

    probs_out = nc.dram_tensor("probs_out", [NQ, K], f32, kind="ExternalOutput").ap()
    acc_out = nc.dram_tensor("acc_out", [128, HID], f32, kind="ExternalOutput").ap()

    KT = N // 128  # 32 key tiles
    f32r = _dt.float32r
    bf16 = _dt.bfloat16

    with tile.TileContext(nc) as tc, ExitStack() as ctx:
        # ---- long-lived SBUF ----
        persist = ctx.enter_context(tc.tile_pool(name="persist", bufs=1))
        featsT = persist.tile([15, N], f32, tag="featsT")
        qHi = persist.tile([HID, NQ], bf16, tag="qHi")
        qLo = persist.tile([HID, NQ], bf16, tag="qLo")
        kHi = persist.tile([HID, N], bf16, tag="kHi")
        kLo = persist.tile([HID, N], bf16, tag="kLo")
        vB = persist.tile([128, KT, HID + 1], bf16, tag="vB")
        iota_s = persist.tile([128, 256], f32, tag="iota")
        acc = persist.tile([128, HID], f32, tag="acc")
        wbig_s = persist.tile([121, 3, WB_COLS], f32, tag="wbig")
        wq_s = persist.tile([15, HID], f32, tag="wq")
        wk_s = persist.tile([15, HID], f32, tag="wk")
        wv_s = persist.tile([15, HID + 1], f32, tag="wv")
        b14_s = persist.tile([128, NGRP], f32, tag="b14")
        id_s = persist.tile([128, 128], f32, tag="id")

        nc.sync.dma_start(wbig_s[:], wbig_d.rearrange("(c p) f -> p c f", c=3))
        nc.sync.dma_start(wq_s[:], wq15_d[:])
        nc.sync.dma_start(wk_s[:], wk15_d[:])
        nc.sync.dma_start(wv_s[:], wv15_d[:])
        nc.sync.dma_start(b14_s[:], bias14_d[:])
        nc.sync.dma_start(id_s[:], ident_d[:])
        nc.sync.dma_start(iota_s[:], iota_d[:])
        nc.sync.dma_start(featsT[14:15, :], ones_d[:])
        nc.vector.memset(acc[:], 0.0)

        xT_view = xT_d.rearrange("(c p) n -> p c n", c=3)

        # ---- stage 1: features for all N keys ----
        with tc.tile_pool(name="f_sb", bufs=3) as fsb, \
             tc.tile_pool(name="f_ps", bufs=2, space="PSUM") as fps, \
             tc.tile_pool(name="f_ps2", bufs=2, space="PSUM") as fps2:
            for t in range(KT):
                xt = fsb.tile([121, 3, 128], f32, tag="xt")
                nc.sync.dma_start(xt[:], xT_view[:, :, t * 128:(t + 1) * 128])
                psF = fps.tile([128, WB_COLS], f32, tag="psF")
                for c in range(3):
                    nc.tensor.matmul(psF[:], xt[:, c, :], wbig_s[:, c, :],
                                     start=(c == 0), stop=(c == 2))
                pooled = fsb.tile([128, NGRP], f32, tag="pooled")
                nc.vector.tensor_reduce(
                    pooled[:], psF[:].rearrange("p (g w) -> p g w", w=GCOL),
                    axis=mybir.AxisListType.X, op=mybir.AluOpType.max)
                feats = fsb.tile([128, NGRP], f32, tag="feats")
                nc.vector.tensor_add(feats[:], pooled[:], b14_s[:])
                nc.vector.tensor_scalar_max(feats[:], feats[:], 0.0)
                psT = fps2.tile([NGRP, 128], f32, tag="psT")
                nc.tensor.transpose(psT[:], feats[:], id_s[:])
                nc.scalar.copy(featsT[0:NGRP, t * 128:(t + 1) * 128], psT[:])

        # ---- stage 2: qT, kT, V ----
        with tc.tile_pool(name="q_ps", bufs=2, space="PSUM") as qps, \
             tc.tile_pool(name="v_ps", bufs=2, space="PSUM") as vps:
            def split_hilo(ps_ap, hi_ap, lo_ap):
                # hi = bf16(x); lo = bf16(x - hi)
                nc.scalar.copy(hi_ap, ps_ap)
                nc.vector.tensor_tensor(lo_ap, ps_ap, hi_ap,
                                        op=mybir.AluOpType.subtract)

            for c in range(NQ // 512):
                ps = qps.tile([HID, 512], f32, tag="psQ")
                nc.tensor.matmul(ps[:], wq_s[:],
                                 featsT[:, c * 512:(c + 1) * 512],
                                 start=True, stop=True)
                split_hilo(ps[:], qHi[:, c * 512:(c + 1) * 512],
                           qLo[:, c * 512:(c + 1) * 512])
            for c in range(N // 512):
                ps = qps.tile([HID, 512], f32, tag="psQ")
                nc.tensor.matmul(ps[:], wk_s[:],
                                 featsT[:, c * 512:(c + 1) * 512],
                                 start=True, stop=True)
                split_hilo(ps[:], kHi[:, c * 512:(c + 1) * 512],
                           kLo[:, c * 512:(c + 1) * 512])
            for c in range(KT):
                ps = vps.tile([128, HID + 1], f32, tag="psV")
                nc.tensor.matmul(ps[:], featsT[:, c * 128:(c + 1) * 128], wv_s[:],
                                 start=True, stop=True)
                nc.scalar.copy(vB[:, c, :], ps[:])

        # ---- stage 3: attention per query tile ----
        with tc.tile_pool(name="e_sb", bufs=4) as esb, \
             tc.tile_pool(name="a_sb", bufs=3) as asb, \
             tc.tile_pool(name="a_small", bufs=3) as asm, \
             tc.tile_pool(name="s_ps", bufs=3, space="PSUM") as sps, \
             tc.tile_pool(name="ct_ps", bufs=2, space="PSUM") as ctps, \
             tc.tile_pool(name="c_ps", bufs=2, space="PSUM") as cps:
            for t in range(QT):
                qs = t * 128
                E = esb.tile([128, N], f32, tag="E")
                for c in range(N // 512):
                    psS = sps.tile([128, 512], f32, tag="psS")
                    ksl = slice(c * 512, (c + 1) * 512)
                    nc.tensor.matmul(psS[:], qHi[:, qs:qs + 128], kHi[:, ksl],
                                     start=True, stop=False)
                    nc.tensor.matmul(psS[:], qHi[:, qs:qs + 128], kLo[:, ksl],
                                     start=False, stop=False)
                    nc.tensor.matmul(psS[:], qLo[:, qs:qs + 128], kHi[:, ksl],
                                     start=False, stop=True)
                    nc.scalar.activation(E[:, c * 512:(c + 1) * 512], psS[:],
                                         mybir.ActivationFunctionType.Exp,
                                         scale=SCALE)

                # candidates: top-8 of each 128-key chunk
                candA = asm.tile([128, NCAND], f32, tag="candA")
                candB = asm.tile([128, NCAND], f32, tag="candB")
                for c in range(CH):
                    nc.vector.max(candA[:, c * 8:(c + 1) * 8],
                                  E[:, c * CHL:(c + 1) * CHL])

                # bitonic merge network over the 32 descending max8 runs
                ALU = mybir.AluOpType
                cur, oth = candA, candB
                for L in (8, 16, 32, 64, 128):
                    sv = cur[:].rearrange("p (b l) -> p b l", l=2 * L)
                    dv = oth[:].rearrange("p (b l) -> p b l", l=2 * L)
                    brev = sv[:, :, 2 * L - 1:L - 1:-1]
                    nc.vector.tensor_tensor(dv[:, :, 0:L], sv[:, :, 0:L],
                                            brev, op=ALU.max)
                    nc.vector.tensor_tensor(dv[:, :, L:2 * L], sv[:, :, 0:L],
                                            brev, op=ALU.min)
                    cur, oth = oth, cur
                    dd = L // 2
                    while dd >= 1:
                        sv = cur[:].rearrange("p (b l) -> p b l", l=2 * dd)
                        dv = oth[:].rearrange("p (b l) -> p b l", l=2 * dd)
                        nc.vector.tensor_tensor(dv[:, :, 0:dd], sv[:, :, 0:dd],
                                                sv[:, :, dd:2 * dd], op=ALU.max)
                        nc.vector.tensor_tensor(dv[:, :, dd:2 * dd],
                                                sv[:, :, 0:dd],
                                                sv[:, :, dd:2 * dd], op=ALU.min)
                        cur, oth = oth, cur
                        dd //= 2
                srt = cur  # [128, 256] fully descending

                # probs = sorted / sum(sorted[:204])
                zs = asm.tile([128, 1], f32, tag="zs")
                rz = asm.tile([128, 1], f32, tag="rz")
                nc.vector.tensor_reduce(zs[:], srt[:, 0:K],
                                        axis=mybir.AxisListType.X,
                                        op=mybir.AluOpType.add)
                nc.vector.reciprocal(rz[:], zs[:])
                probs = asm.tile([128, K], f32, tag="probs")
                nc.scalar.activation(probs[:], srt[:, 0:K],
                                     mybir.ActivationFunctionType.Copy,
                                     scale=rz[:])
                nc.sync.dma_start(probs_out[qs:qs + 128, :], probs[:])

                # --- secant-Newton count-corrected mask threshold ---
                ALU = mybir.AluOpType
                AF = mybir.ActivationFunctionType
                Pbf = asb.tile([128, N], bf16, tag="Pbf")

                def count_ge(tau_ap, r_out):
                    # R = count(E >= tau) = (sum(sign(E - tau)) + 4097) / 2
                    ntau = asm.tile([128, 1], f32, tag="ntau")
                    nc.vector.tensor_scalar_mul(ntau[:], tau_ap, -1.0)
                    sgn = asm.tile([128, 1], f32, tag="sgn")
                    nc.scalar.activation(Pbf[:], E[:], AF.Sign,
                                         bias=ntau[:], accum_out=sgn[:])
                    nc.vector.tensor_scalar(r_out, sgn[:], 0.5, 2048.5,
                                            op0=ALU.mult, op1=ALU.add)

                def window_extract(j_ap, tau_out):
                    # tau = srt[j] (lerped for fractional/half-integer j)
                    jneg = asm.tile([128, 1], f32, tag="jneg")
                    nc.vector.tensor_scalar_mul(jneg[:], j_ap, -1.0)
                    w208 = asm.tile([128, NCAND], f32, tag="w208")
                    nc.scalar.activation(w208[:], iota_s[:],
                                         AF.Abs, bias=jneg[:])
                    nc.scalar.activation(w208[:], w208[:], AF.Relu,
                                         scale=-1.0, bias=1.0)
                    dummy = asm.tile([128, NCAND], f32, tag="dummy")
                    nc.vector.scalar_tensor_tensor(
                        dummy[:], w208[:], 0.0, srt[:],
                        op0=ALU.add, op1=ALU.mult, accum_out=tau_out)

                # two-probe secant: R0 at srt[203], Rb at srt[185]
                JP = 185
                r0 = asm.tile([128, 1], f32, tag="r0")
                count_ge(srt[:, K - 1:K], r0[:])
                rb = asm.tile([128, 1], f32, tag="rb")
                count_ge(srt[:, JP:JP + 1], rb[:])
                den = asm.tile([128, 1], f32, tag="den")
                nc.vector.tensor_tensor(den[:], r0[:], rb[:], op=ALU.subtract)
                nc.vector.tensor_scalar_max(den[:], den[:], float(203 - JP))
                nc.vector.reciprocal(den[:], den[:])
                step = asm.tile([128, 1], f32, tag="step")
                nc.vector.tensor_scalar(step[:], r0[:], -205.0, float(203 - JP),
                                        op0=ALU.add, op1=ALU.mult)
                nc.vector.tensor_tensor(step[:], step[:], den[:], op=ALU.mult)
                j2 = asm.tile([128, 1], f32, tag="j2")
                nc.vector.tensor_scalar(step[:], step[:], -1.0, 203.0,
                                        op0=ALU.mult, op1=ALU.add)
                nc.vector.tensor_scalar(j2[:], step[:], 100.0, 255.0,
                                        op0=ALU.max, op1=ALU.min)
                tau2 = asm.tile([128, 1], f32, tag="tau2")
                window_extract(j2[:], tau2[:])
                ntau2 = asm.tile([128, 1], f32, tag="ntau2")
                nc.vector.tensor_scalar_mul(ntau2[:], tau2[:], -1.0)

                # blockwise: mask (ACT+DVE+Pool) -> xbar transpose -> ctx MM
                NB = 4
                BW = N // NB          # 1024 columns per block
                BC = KT // NB         # 8 key chunks per block
                mbf = asb.tile([128, N], bf16, tag="mbf")
                PT = asb.tile([128, KT, 128], bf16, tag="PT")
                psCt = ctps.tile([HID + 1, 128], f32, tag="psCt")
                for blk in range(NB):
                    csl = slice(blk * BW, (blk + 1) * BW)
                    nc.scalar.activation(mbf[:, csl], E[:, csl], AF.Sign,
                                         bias=ntau2[:])
                    nc.vector.tensor_scalar_max(mbf[:, csl], mbf[:, csl], 0.0)
                    nc.gpsimd.tensor_tensor(Pbf[:, csl], E[:, csl],
                                            mbf[:, csl], op=ALU.mult)
                    nc.sync.dma_start(PT[:, blk * BC:(blk + 1) * BC, :],
                                      Pbf[:, csl], transpose=True)
                    for cc in range(BC):
                        c = blk * BC + cc
                        nc.tensor.matmul(psCt[:], vB[:, c, :], PT[:, c, :],
                                         start=(c == 0), stop=(c == KT - 1))
                ctxT = asm.tile([HID + 1, 128], f32, tag="ctxT")
                nc.scalar.copy(ctxT[:], psCt[:])
                psC2 = cps.tile([128, HID + 1], f32, tag="psC2")
                nc.tensor.transpose(psC2[:], ctxT[:], id_s[0:HID + 1, 0:HID + 1])
                rzd = asm.tile([128, 1], f32, tag="rzd")
                nc.vector.reciprocal(rzd[:], psC2[:, HID:HID + 1])
                ctxn = asm.tile([128, HID], f32, tag="ctxn")
                nc.scalar.activation(ctxn[:], psC2[:, 0:HID],
                                     mybir.ActivationFunctionType.Copy,
                                     scale=rzd[:])
                nc.vector.tensor_add(acc[:], acc[:], ctxn[:])

            nc.sync.dma_start(acc_out[:], acc[:])

    nc.compile()
    return nc


_NC_CACHE = None


def _get_nc():
    global _NC_CACHE
    if _NC_CACHE is None:
        _NC_CACHE = build_nc()
    return _NC_CACHE


def _host_inputs(inputs):
    """Build per-core input maps."""
    x = np.asarray(inputs["x"], np.float32)
    wbig = _build_wbig(inputs)

    def wxx15(w, b, extra_ones=False):
        o = w.shape[0]
        cols = o + (1 if extra_ones else 0)
        m = np.zeros((15, cols), np.float32)
        m[0:AFN, 0:o] = np.asarray(w, np.float32).T
        m[14, 0:o] = np.asarray(b, np.float32)
        if extra_ones:
            m[14, o] = 1.0
        return m

    wq15 = wxx15(inputs["w_q"], inputs["b_q"])
    wk15 = wxx15(inputs["w_k"], inputs["b_k"])
    wv15 = wxx15(inputs["w_v"], inputs["b_v"], extra_ones=True)
    bias14 = np.broadcast_to(
        np.concatenate([np.asarray(inputs["conv_b%d" % h], np.float32)
                        for h in KS])[None, :], (128, NGRP)).copy()
    ident = np.eye(128, dtype=np.float32)

    in_maps = []
    for core in range(8):
        b, h = core // 2, core % 2
        xb = x[b].reshape(N, XROWS)
        if h == 1:
            xb = np.concatenate([xb[NQ:], xb[:NQ]], axis=0)
        xT = np.empty((XR_PAD, N), np.float32)
        xT[0:XROWS] = xb.T
        xT[XROWS] = 1.0
        xT[XROWS + 1:] = 0.0
        in_maps.append({
            "xT": np.ascontiguousarray(xT),
            "wbig": wbig, "wq15": wq15, "wk15": wk15, "wv15": wv15,
            "bias14": bias14, "ident": ident,
            "ones_row": np.ones((1, N), np.float32),
            "iota_row": np.broadcast_to(
                np.arange(256, dtype=np.float32)[None, :], (128, 256)).copy(),
        })
    return in_maps


def kernel(**inputs):
    nc = _get_nc()
    in_maps = _host_inputs(inputs)
    res = run_bass_kernel_spmd(nc, in_maps, core_ids=list(range(8)))
    return _assemble(inputs, res.results)


def _assemble(inputs, results):
    probs = np.empty((B, N, K), np.float32)
    ctx_sum = np.zeros((B, HID), np.float64)
    for core in range(8):
        b, h = core // 2, core % 2
        r = results[core]
        probs[b, h * NQ:(h + 1) * NQ] = r["probs_out"]
        ctx_sum[b] += r["acc_out"].sum(axis=0, dtype=np.float64)

    w_attn = np.asarray(inputs["w_attn"], np.float64)
    b_attn = np.asarray(inputs["b_attn"], np.float64)
    w_mil = np.asarray(inputs["w_mil"], np.float64)
    b_mil = np.asarray(inputs["b_mil"], np.float64)
    wc = w_mil @ w_attn                     # [2, 10]
    bc = b_attn @ w_mil.T + b_mil           # [2]
    pooled = (ctx_sum @ wc.T) / N + bc
    return (pooled.astype(np.float32),
            probs.reshape(B, 1, N, 1, K).astype(np.float32))
